# revision 1
# baseline (speedup 1.0000x reference)
"""FFM layer kernel for Trainium2 (8 NeuronCores, data-parallel over batch).

Math (reference):
  idx[b,j]  = 13 + j*10000 + sparse_x[b,j]                 (26 sparse fields)
  linear    = dense_x @ w[:13] + sum_j w[idx] + w0         (B,1)
  field_f   = einsum('bd,dfk', dense_x, v[:13]) + sum_j v[idx]   (B,39,8)
  s         = sum_f field_f                                 (B,8)
  cross     = 0.5*(sum_k s^2 - sum_{f,k} field_f^2)
  out       = sigmoid(linear + cross)

Device strategy (per core, 512 samples):
  - Table rows augmented+padded on host: row r = [v[r].flat(312), w[r], 0...]
    (320 f32 = 1280 B, 64B-aligned, %256==0 for dma_gather).
  - 26 dma_gather ops (one per sparse field, 512 int16 indices each) fetch rows
    from the field's 10000-row table slice into [128, 4, 320] tiles; sample s
    lands at [s%128, s//128, :].  DVE accumulates each landed tile into a
    [128, 4*320] running sum, overlapped with the next gather's transfer.
  - Dense contribution (and the linear term + w0) via one PE matmul per chunk:
    lhsT = dense_x^T chunk padded to [128,128] with an all-ones row 13,
    rhs[d] = [v[d].flat, w[d], 0...] and rhs[13,312] = w0.
  - DVE: strided reductions for sum_f, fused square-reduce for the cross term.
  - ACT: sigmoid.
"""

import numpy as np

N_DENSE = 13
N_SPARSE = 26
VOCAB = 10000
N_FIELD = 39
N_FEAT = N_DENSE + N_SPARSE * VOCAB  # 260013
K = 8
ROW = N_FIELD * K  # 312
ROWP = 320  # padded row (f32 elems) -> 1280 B
BATCH = 4096
N_CORES = 8
BC = BATCH // N_CORES  # 512 per core
P = 128
NCHUNK = BC // P  # 4
IDXC = BC // 16  # 32 int16 index columns per field

_CACHE: dict = {}


def _build_program():
    import concourse.bacc as bacc
    import concourse.tile as tile
    import concourse.mybir as mybir

    f32 = mybir.dt.float32
    i16 = mybir.dt.int16

    nc = bacc.Bacc("TRN2", target_bir_lowering=False, debug=False)

    table = nc.dram_tensor("table", [N_FEAT, ROWP], f32, kind="ExternalInput")
    xt = nc.dram_tensor("xt", [P, BC], f32, kind="ExternalInput")
    vd = nc.dram_tensor("vd", [P, ROWP], f32, kind="ExternalInput")
    idx16 = nc.dram_tensor("idx16", [P, N_SPARSE * IDXC], i16, kind="ExternalInput")
    out = nc.dram_tensor("out", [BC, 1], f32, kind="ExternalOutput")

    with tile.TileContext(nc) as tc:
        with (
            tc.tile_pool(name="const", bufs=1) as cpool,
            tc.tile_pool(name="gather", bufs=4) as gpool,
            tc.tile_pool(name="work", bufs=2) as wpool,
            tc.tile_pool(name="psum", bufs=2, space="PSUM") as ppool,
        ):
            xt_sb = cpool.tile([P, BC], f32)
            nc.sync.dma_start(out=xt_sb[:], in_=xt[:])
            vd_sb = cpool.tile([P, ROWP], f32)
            nc.sync.dma_start(out=vd_sb[:], in_=vd[:])
            idx_sb = cpool.tile([P, N_SPARSE * IDXC], i16)
            nc.sync.dma_start(out=idx_sb[:], in_=idx16[:])

            # running sum over fields of the gathered rows, all 4 chunks wide
            acc = cpool.tile([P, NCHUNK * ROWP], f32)

            for j in range(N_SPARSE):
                g = gpool.tile([P, NCHUNK, ROWP], f32, tag="g")
                base = N_DENSE + j * VOCAB
                import os as _os

                _sp = _os.environ.get("K_SINGLE_PACKET", "1") == "1"
                _nq = int(_os.environ.get("K_NQUEUES", "1"))
                nc.gpsimd.dma_gather(
                    out_ap=g[:],
                    in_ap=table[base:base + VOCAB, :],
                    idxs_ap=idx_sb[:, j * IDXC:(j + 1) * IDXC],
                    num_idxs=BC,
                    num_idxs_reg=BC,
                    elem_size=ROWP,
                    single_packet=_sp,
                    queue_num=j % _nq,
                )
                gf = g[:].rearrange("p c m -> p (c m)")
                if j == 0:
                    nc.vector.tensor_copy(out=acc[:], in_=gf)
                else:
                    nc.vector.tensor_tensor(
                        out=acc[:], in0=acc[:], in1=gf, op=mybir.AluOpType.add
                    )

            for c in range(NCHUNK):
                psum = ppool.tile([P, ROWP], f32, space="PSUM")
                nc.tensor.matmul(
                    out=psum[:],
                    lhsT=xt_sb[:, c * P:(c + 1) * P],
                    rhs=vd_sb[:],
                    start=True,
                    stop=True,
                )

                field = wpool.tile([P, ROWP], f32)
                nc.vector.tensor_tensor(
                    out=field[:],
                    in0=acc[:, c * ROWP:(c + 1) * ROWP],
                    in1=psum[:],
                    op=mybir.AluOpType.add,
                )

                # q = sum(field[:, :312]^2)   (square on ACT, reduce on DVE)
                sq = wpool.tile([P, ROW], f32)
                nc.scalar.square(sq[:], field[:, :ROW])
                q = wpool.tile([P, 1], f32)
                nc.vector.reduce_sum(out=q[:], in_=sq[:], axis=mybir.AxisListType.X)

                # s[k] = sum_f field[f*8+k]: view [P, K, N_FIELD], reduce X
                s = wpool.tile([P, K], f32)
                fv = field[:, :ROW].rearrange("p (f k) -> p k f", f=N_FIELD)
                nc.vector.reduce_sum(out=s[:], in_=fv, axis=mybir.AxisListType.X)

                ss = wpool.tile([P, K], f32)
                nc.vector.tensor_tensor(
                    out=ss[:], in0=s[:], in1=s[:], op=mybir.AluOpType.mult
                )
                ssum = wpool.tile([P, 1], f32)
                nc.vector.reduce_sum(
                    out=ssum[:], in_=ss[:], axis=mybir.AxisListType.X
                )
                d = wpool.tile([P, 1], f32)
                nc.vector.tensor_tensor(
                    out=d[:], in0=ssum[:], in1=q[:], op=mybir.AluOpType.subtract
                )

                # out = sigmoid(0.5*d + linear)  (linear incl. w0 = field col 312)
                oc = wpool.tile([P, 1], f32)
                nc.scalar.activation(
                    oc[:],
                    d[:],
                    mybir.ActivationFunctionType.Sigmoid,
                    bias=field[:, ROW:ROW + 1],
                    scale=0.5,
                )
                nc.sync.dma_start(out=out[c * P:(c + 1) * P, :], in_=oc[:])

    nc.compile()
    return nc


def _prep_inputs(dense_x, sparse_x, w0, w, v):
    table = np.zeros((N_FEAT, ROWP), dtype=np.float32)
    table[:, :ROW] = v.reshape(N_FEAT, ROW)
    table[:, ROW] = w[:, 0]

    vd = np.zeros((P, ROWP), dtype=np.float32)
    vd[:N_DENSE, :ROW] = v[:N_DENSE].reshape(N_DENSE, ROW)
    vd[:N_DENSE, ROW] = w[:N_DENSE, 0]
    vd[N_DENSE, ROW] = np.float32(w0[0])

    xt_full = np.zeros((P, BATCH), dtype=np.float32)
    xt_full[:N_DENSE] = dense_x.T
    xt_full[N_DENSE] = 1.0

    in_maps = []
    for r in range(N_CORES):
        b0 = r * BC
        sp = sparse_x[b0:b0 + BC].astype(np.int16)  # values < 10000 fit
        idx16 = np.zeros((P, N_SPARSE * IDXC), dtype=np.int16)
        for j in range(N_SPARSE):
            # gather position g reads idx[g%16, g//16]; g == sample index.
            # Replicated across all 8 GPSIMD-core partition groups (HW reads
            # its own 16-partition window).
            blk = sp[:, j].reshape(IDXC, 16).T
            idx16[:, j * IDXC:(j + 1) * IDXC] = np.tile(blk, (P // 16, 1))
        in_maps.append(
            {
                "table": table,
                "xt": np.ascontiguousarray(xt_full[:, b0:b0 + BC]),
                "vd": vd,
                "idx16": idx16,
            }
        )
    return in_maps


def kernel(dense_x, sparse_x, w0, w, v, _trace=False, _trace_kwargs=None):
    from concourse.bass_utils import run_bass_kernel_spmd

    if "nc" not in _CACHE:
        _CACHE["nc"] = _build_program()
    nc = _CACHE["nc"]

    in_maps = _prep_inputs(dense_x, sparse_x, w0, w, v)
    kw = {}
    if _trace:
        kw["trace"] = True
        if _trace_kwargs:
            kw.update(_trace_kwargs)
    res = run_bass_kernel_spmd(nc, in_maps, core_ids=list(range(N_CORES)), **kw)
    outs = [res.results[r]["out"] for r in range(N_CORES)]
    full = np.concatenate(outs, axis=0).astype(np.float32)
    if _trace:
        _CACHE["last_exec_time_ns"] = res.exec_time_ns
        _CACHE["last_results"] = res
    return full



# revision 3
# speedup vs baseline: 1.4245x; 1.4245x over previous
"""FFM layer kernel for Trainium2 (8 NeuronCores, data-parallel over batch).

Math (reference):
  idx[b,j]  = 13 + j*10000 + sparse_x[b,j]                 (26 sparse fields)
  linear    = dense_x @ w[:13] + sum_j w[idx] + w0         (B,1)
  field_f   = einsum('bd,dfk', dense_x, v[:13]) + sum_j v[idx]   (B,39,8)
  s         = sum_f field_f                                 (B,8)
  cross     = 0.5*(sum_k s^2 - sum_{f,k} field_f^2)
  out       = sigmoid(linear + cross)

Device strategy (per core, 512 samples):
  - Table rows augmented+padded on host: row r = [v[r].flat(312), w[r], 0...]
    (320 f32 = 1280 B, 64B-aligned, %256==0 for dma_gather).
  - 26 dma_gather ops (one per sparse field, 512 int16 indices each) fetch rows
    from the field's 10000-row table slice into [128, 4, 320] tiles; sample s
    lands at [s%128, s//128, :].  DVE accumulates each landed tile into a
    [128, 4*320] running sum, overlapped with the next gather's transfer.
  - Dense contribution (and the linear term + w0) via one PE matmul per chunk:
    lhsT = dense_x^T chunk padded to [128,128] with an all-ones row 13,
    rhs[d] = [v[d].flat, w[d], 0...] and rhs[13,312] = w0.
  - DVE: strided reductions for sum_f, fused square-reduce for the cross term.
  - ACT: sigmoid.
"""

import numpy as np

N_DENSE = 13
N_SPARSE = 26
VOCAB = 10000
N_FIELD = 39
N_FEAT = N_DENSE + N_SPARSE * VOCAB  # 260013
K = 8
ROW = N_FIELD * K  # 312
ROWP = 320  # padded row (f32 elems) -> 1280 B
BATCH = 4096
N_CORES = 8
BC = BATCH // N_CORES  # 512 per core
P = 128
NCHUNK = BC // P  # 4
IDXC = BC // 16  # 32 int16 index columns per field

_CACHE: dict = {}


def _build_program():
    import concourse.bacc as bacc
    import concourse.tile as tile
    import concourse.mybir as mybir

    f32 = mybir.dt.float32
    i16 = mybir.dt.int16

    import os as _os

    _nq = int(_os.environ.get("K_NQUEUES", "4"))
    nc = bacc.Bacc(
        "TRN2", target_bir_lowering=False, debug=False, num_swdge_queues=_nq
    )

    table = nc.dram_tensor("table", [N_FEAT, ROWP], f32, kind="ExternalInput")
    xt = nc.dram_tensor("xt", [P, BC], f32, kind="ExternalInput")
    vd = nc.dram_tensor("vd", [P, ROWP], f32, kind="ExternalInput")
    idx16 = nc.dram_tensor("idx16", [P, N_SPARSE * IDXC], i16, kind="ExternalInput")
    out = nc.dram_tensor("out", [BC, 1], f32, kind="ExternalOutput")

    with tile.TileContext(nc) as tc:
        with (
            tc.tile_pool(name="const", bufs=1) as cpool,
            tc.tile_pool(name="gather", bufs=4) as gpool,
            tc.tile_pool(name="work", bufs=2) as wpool,
            tc.tile_pool(name="psum", bufs=2, space="PSUM") as ppool,
        ):
            xt_sb = cpool.tile([P, BC], f32)
            nc.sync.dma_start(out=xt_sb[:], in_=xt[:])
            vd_sb = cpool.tile([P, ROWP], f32)
            nc.sync.dma_start(out=vd_sb[:], in_=vd[:])
            idx_sb = cpool.tile([P, N_SPARSE * IDXC], i16)
            nc.sync.dma_start(out=idx_sb[:], in_=idx16[:])

            # running sum over fields of the gathered rows, all 4 chunks wide
            acc = cpool.tile([P, NCHUNK * ROWP], f32)

            for j in range(N_SPARSE):
                g = gpool.tile([P, NCHUNK, ROWP], f32, tag="g")
                base = N_DENSE + j * VOCAB

                _sp = _os.environ.get("K_SINGLE_PACKET", "1") == "1"
                nc.gpsimd.dma_gather(
                    out_ap=g[:],
                    in_ap=table[base:base + VOCAB, :],
                    idxs_ap=idx_sb[:, j * IDXC:(j + 1) * IDXC],
                    num_idxs=BC,
                    num_idxs_reg=BC,
                    elem_size=ROWP,
                    single_packet=_sp,
                    queue_num=j % _nq,
                )
                gf = g[:].rearrange("p c m -> p (c m)")
                if j == 0:
                    nc.vector.tensor_copy(out=acc[:], in_=gf)
                else:
                    nc.vector.tensor_tensor(
                        out=acc[:], in0=acc[:], in1=gf, op=mybir.AluOpType.add
                    )

            for c in range(NCHUNK):
                psum = ppool.tile([P, ROWP], f32, space="PSUM")
                nc.tensor.matmul(
                    out=psum[:],
                    lhsT=xt_sb[:, c * P:(c + 1) * P],
                    rhs=vd_sb[:],
                    start=True,
                    stop=True,
                )

                field = wpool.tile([P, ROWP], f32)
                nc.vector.tensor_tensor(
                    out=field[:],
                    in0=acc[:, c * ROWP:(c + 1) * ROWP],
                    in1=psum[:],
                    op=mybir.AluOpType.add,
                )

                # q = sum(field[:, :312]^2)   (square on ACT, reduce on DVE)
                sq = wpool.tile([P, ROW], f32)
                nc.scalar.square(sq[:], field[:, :ROW])
                q = wpool.tile([P, 1], f32)
                nc.vector.reduce_sum(out=q[:], in_=sq[:], axis=mybir.AxisListType.X)

                # s[k] = sum_f field[f*8+k]: view [P, K, N_FIELD], reduce X
                s = wpool.tile([P, K], f32)
                fv = field[:, :ROW].rearrange("p (f k) -> p k f", f=N_FIELD)
                nc.vector.reduce_sum(out=s[:], in_=fv, axis=mybir.AxisListType.X)

                ss = wpool.tile([P, K], f32)
                nc.vector.tensor_tensor(
                    out=ss[:], in0=s[:], in1=s[:], op=mybir.AluOpType.mult
                )
                ssum = wpool.tile([P, 1], f32)
                nc.vector.reduce_sum(
                    out=ssum[:], in_=ss[:], axis=mybir.AxisListType.X
                )
                d = wpool.tile([P, 1], f32)
                nc.vector.tensor_tensor(
                    out=d[:], in0=ssum[:], in1=q[:], op=mybir.AluOpType.subtract
                )

                # out = sigmoid(0.5*d + linear)  (linear incl. w0 = field col 312)
                oc = wpool.tile([P, 1], f32)
                nc.scalar.activation(
                    oc[:],
                    d[:],
                    mybir.ActivationFunctionType.Sigmoid,
                    bias=field[:, ROW:ROW + 1],
                    scale=0.5,
                )
                nc.sync.dma_start(out=out[c * P:(c + 1) * P, :], in_=oc[:])

    nc.compile()
    return nc


def _prep_inputs(dense_x, sparse_x, w0, w, v):
    table = np.zeros((N_FEAT, ROWP), dtype=np.float32)
    table[:, :ROW] = v.reshape(N_FEAT, ROW)
    table[:, ROW] = w[:, 0]

    vd = np.zeros((P, ROWP), dtype=np.float32)
    vd[:N_DENSE, :ROW] = v[:N_DENSE].reshape(N_DENSE, ROW)
    vd[:N_DENSE, ROW] = w[:N_DENSE, 0]
    vd[N_DENSE, ROW] = np.float32(w0[0])

    xt_full = np.zeros((P, BATCH), dtype=np.float32)
    xt_full[:N_DENSE] = dense_x.T
    xt_full[N_DENSE] = 1.0

    in_maps = []
    for r in range(N_CORES):
        b0 = r * BC
        sp = sparse_x[b0:b0 + BC].astype(np.int16)  # values < 10000 fit
        idx16 = np.zeros((P, N_SPARSE * IDXC), dtype=np.int16)
        for j in range(N_SPARSE):
            # gather position g reads idx[g%16, g//16]; g == sample index.
            # Replicated across all 8 GPSIMD-core partition groups (HW reads
            # its own 16-partition window).
            blk = sp[:, j].reshape(IDXC, 16).T
            idx16[:, j * IDXC:(j + 1) * IDXC] = np.tile(blk, (P // 16, 1))
        in_maps.append(
            {
                "table": table,
                "xt": np.ascontiguousarray(xt_full[:, b0:b0 + BC]),
                "vd": vd,
                "idx16": idx16,
            }
        )
    return in_maps


def kernel(dense_x, sparse_x, w0, w, v, _trace=False, _trace_kwargs=None):
    from concourse.bass_utils import run_bass_kernel_spmd

    if "nc" not in _CACHE:
        _CACHE["nc"] = _build_program()
    nc = _CACHE["nc"]

    in_maps = _prep_inputs(dense_x, sparse_x, w0, w, v)
    kw = {}
    if _trace:
        kw["trace"] = True
        if _trace_kwargs:
            kw.update(_trace_kwargs)
    res = run_bass_kernel_spmd(nc, in_maps, core_ids=list(range(N_CORES)), **kw)
    outs = [res.results[r]["out"] for r in range(N_CORES)]
    full = np.concatenate(outs, axis=0).astype(np.float32)
    if _trace:
        _CACHE["last_exec_time_ns"] = res.exec_time_ns
        _CACHE["last_results"] = res
    return full



# revision 5
# speedup vs baseline: 1.7215x; 1.2085x over previous
"""FFM layer kernel for Trainium2 (8 NeuronCores, data-parallel over batch).

Math (reference):
  idx[b,j]  = 13 + j*10000 + sparse_x[b,j]                 (26 sparse fields)
  linear    = dense_x @ w[:13] + sum_j w[idx] + w0         (B,1)
  field_f   = einsum('bd,dfk', dense_x, v[:13]) + sum_j v[idx]   (B,39,8)
  s         = sum_f field_f                                 (B,8)
  cross     = 0.5*(sum_k s^2 - sum_{f,k} field_f^2)
  out       = sigmoid(linear + cross)

Device strategy (per core, 512 samples):
  - Table rows augmented+padded on host: row r = [v[r].flat(312), w[r], 0...]
    (320 f32 = 1280 B, 64B-aligned, %256==0 for dma_gather).
  - 26 dma_gather ops (one per sparse field, 512 int16 indices each) fetch rows
    from the field's 10000-row table slice into [128, 4, 320] tiles; sample s
    lands at [s%128, s//128, :].  DVE accumulates each landed tile into a
    [128, 4*320] running sum, overlapped with the next gather's transfer.
  - Dense contribution (and the linear term + w0) via one PE matmul per chunk:
    lhsT = dense_x^T chunk padded to [128,128] with an all-ones row 13,
    rhs[d] = [v[d].flat, w[d], 0...] and rhs[13,312] = w0.
  - DVE: strided reductions for sum_f, fused square-reduce for the cross term.
  - ACT: sigmoid.
"""

import numpy as np

N_DENSE = 13
N_SPARSE = 26
VOCAB = 10000
N_FIELD = 39
N_FEAT = N_DENSE + N_SPARSE * VOCAB  # 260013
K = 8
ROW = N_FIELD * K  # 312
ROWP = 320  # padded row (f32 elems) -> 1280 B
BATCH = 4096
N_CORES = 8
BC = BATCH // N_CORES  # 512 per core
P = 128
NCHUNK = BC // P  # 4
IDXC = BC // 16  # 32 int16 index columns per field

_CACHE: dict = {}


def _build_program():
    import concourse.bacc as bacc
    import concourse.tile as tile
    import concourse.mybir as mybir

    f32 = mybir.dt.float32
    i16 = mybir.dt.int16

    import os as _os

    _nq = int(_os.environ.get("K_NQUEUES", "4"))
    nc = bacc.Bacc(
        "TRN2", target_bir_lowering=False, debug=False, num_swdge_queues=_nq
    )

    table = nc.dram_tensor("table", [N_FEAT, ROWP], f32, kind="ExternalInput")
    xt = nc.dram_tensor("xt", [P, BC], f32, kind="ExternalInput")
    vd = nc.dram_tensor("vd", [P, ROWP], f32, kind="ExternalInput")
    idx16 = nc.dram_tensor("idx16", [P, N_SPARSE * IDXC], i16, kind="ExternalInput")
    out = nc.dram_tensor("out", [BC, 1], f32, kind="ExternalOutput")

    with tile.TileContext(nc) as tc:
        with (
            tc.tile_pool(name="const", bufs=1) as cpool,
            tc.tile_pool(name="gather", bufs=N_SPARSE) as gpool,
            tc.tile_pool(name="work", bufs=2) as wpool,
            tc.tile_pool(name="psum", bufs=2, space="PSUM") as ppool,
        ):
            xt_sb = cpool.tile([P, BC], f32)
            nc.sync.dma_start(out=xt_sb[:], in_=xt[:])
            vd_sb = cpool.tile([P, ROWP], f32)
            nc.sync.dma_start(out=vd_sb[:], in_=vd[:])
            idx_sb = cpool.tile([P, N_SPARSE * IDXC], i16)
            nc.sync.dma_start(out=idx_sb[:], in_=idx16[:])

            # per-queue running sums over fields of the gathered rows
            accs = [
                cpool.tile([P, NCHUNK * ROWP], f32, name=f"acc{q}")
                for q in range(_nq)
            ]

            _sp = _os.environ.get("K_SINGLE_PACKET", "1") == "1"
            for j in range(N_SPARSE):
                g = gpool.tile([P, NCHUNK, ROWP], f32, tag="g")
                base = N_DENSE + j * VOCAB
                q = j % _nq

                nc.gpsimd.dma_gather(
                    out_ap=g[:],
                    in_ap=table[base:base + VOCAB, :],
                    idxs_ap=idx_sb[:, j * IDXC:(j + 1) * IDXC],
                    num_idxs=BC,
                    num_idxs_reg=BC,
                    elem_size=ROWP,
                    single_packet=_sp,
                    queue_num=q,
                )
                gf = g[:].rearrange("p c m -> p (c m)")
                if j < _nq:
                    nc.vector.tensor_copy(out=accs[q][:], in_=gf)
                else:
                    nc.vector.tensor_tensor(
                        out=accs[q][:], in0=accs[q][:], in1=gf,
                        op=mybir.AluOpType.add,
                    )

            # combine the per-queue partial sums into accs[0]
            if _nq > 1:
                if _nq == 4:
                    nc.vector.tensor_tensor(
                        out=accs[0][:], in0=accs[0][:], in1=accs[1][:],
                        op=mybir.AluOpType.add,
                    )
                    nc.vector.tensor_tensor(
                        out=accs[2][:], in0=accs[2][:], in1=accs[3][:],
                        op=mybir.AluOpType.add,
                    )
                    nc.vector.tensor_tensor(
                        out=accs[0][:], in0=accs[0][:], in1=accs[2][:],
                        op=mybir.AluOpType.add,
                    )
                else:
                    for q in range(1, _nq):
                        nc.vector.tensor_tensor(
                            out=accs[0][:], in0=accs[0][:], in1=accs[q][:],
                            op=mybir.AluOpType.add,
                        )
            acc = accs[0]

            for c in range(NCHUNK):
                psum = ppool.tile([P, ROWP], f32, space="PSUM")
                nc.tensor.matmul(
                    out=psum[:],
                    lhsT=xt_sb[:, c * P:(c + 1) * P],
                    rhs=vd_sb[:],
                    start=True,
                    stop=True,
                )

                field = wpool.tile([P, ROWP], f32)
                nc.vector.tensor_tensor(
                    out=field[:],
                    in0=acc[:, c * ROWP:(c + 1) * ROWP],
                    in1=psum[:],
                    op=mybir.AluOpType.add,
                )

                # q = sum(field[:, :312]^2)   (square on ACT, reduce on DVE)
                sq = wpool.tile([P, ROW], f32)
                nc.scalar.square(sq[:], field[:, :ROW])
                q = wpool.tile([P, 1], f32)
                nc.vector.reduce_sum(out=q[:], in_=sq[:], axis=mybir.AxisListType.X)

                # s[k] = sum_f field[f*8+k]: view [P, K, N_FIELD], reduce X
                s = wpool.tile([P, K], f32)
                fv = field[:, :ROW].rearrange("p (f k) -> p k f", f=N_FIELD)
                nc.vector.reduce_sum(out=s[:], in_=fv, axis=mybir.AxisListType.X)

                ss = wpool.tile([P, K], f32)
                nc.vector.tensor_tensor(
                    out=ss[:], in0=s[:], in1=s[:], op=mybir.AluOpType.mult
                )
                ssum = wpool.tile([P, 1], f32)
                nc.vector.reduce_sum(
                    out=ssum[:], in_=ss[:], axis=mybir.AxisListType.X
                )
                d = wpool.tile([P, 1], f32)
                nc.vector.tensor_tensor(
                    out=d[:], in0=ssum[:], in1=q[:], op=mybir.AluOpType.subtract
                )

                # out = sigmoid(0.5*d + linear)  (linear incl. w0 = field col 312)
                oc = wpool.tile([P, 1], f32)
                nc.scalar.activation(
                    oc[:],
                    d[:],
                    mybir.ActivationFunctionType.Sigmoid,
                    bias=field[:, ROW:ROW + 1],
                    scale=0.5,
                )
                nc.sync.dma_start(out=out[c * P:(c + 1) * P, :], in_=oc[:])

    nc.compile()
    return nc


def _prep_inputs(dense_x, sparse_x, w0, w, v):
    table = np.zeros((N_FEAT, ROWP), dtype=np.float32)
    table[:, :ROW] = v.reshape(N_FEAT, ROW)
    table[:, ROW] = w[:, 0]

    vd = np.zeros((P, ROWP), dtype=np.float32)
    vd[:N_DENSE, :ROW] = v[:N_DENSE].reshape(N_DENSE, ROW)
    vd[:N_DENSE, ROW] = w[:N_DENSE, 0]
    vd[N_DENSE, ROW] = np.float32(w0[0])

    xt_full = np.zeros((P, BATCH), dtype=np.float32)
    xt_full[:N_DENSE] = dense_x.T
    xt_full[N_DENSE] = 1.0

    in_maps = []
    for r in range(N_CORES):
        b0 = r * BC
        sp = sparse_x[b0:b0 + BC].astype(np.int16)  # values < 10000 fit
        idx16 = np.zeros((P, N_SPARSE * IDXC), dtype=np.int16)
        for j in range(N_SPARSE):
            # gather position g reads idx[g%16, g//16]; g == sample index.
            # Replicated across all 8 GPSIMD-core partition groups (HW reads
            # its own 16-partition window).
            blk = sp[:, j].reshape(IDXC, 16).T
            idx16[:, j * IDXC:(j + 1) * IDXC] = np.tile(blk, (P // 16, 1))
        in_maps.append(
            {
                "table": table,
                "xt": np.ascontiguousarray(xt_full[:, b0:b0 + BC]),
                "vd": vd,
                "idx16": idx16,
            }
        )
    return in_maps


def kernel(dense_x, sparse_x, w0, w, v, _trace=False, _trace_kwargs=None):
    from concourse.bass_utils import run_bass_kernel_spmd

    if "nc" not in _CACHE:
        _CACHE["nc"] = _build_program()
    nc = _CACHE["nc"]

    in_maps = _prep_inputs(dense_x, sparse_x, w0, w, v)
    kw = {}
    if _trace:
        kw["trace"] = True
        if _trace_kwargs:
            kw.update(_trace_kwargs)
    res = run_bass_kernel_spmd(nc, in_maps, core_ids=list(range(N_CORES)), **kw)
    outs = [res.results[r]["out"] for r in range(N_CORES)]
    full = np.concatenate(outs, axis=0).astype(np.float32)
    if _trace:
        _CACHE["last_exec_time_ns"] = res.exec_time_ns
        _CACHE["last_results"] = res
    return full



# revision 6
# speedup vs baseline: 2.0409x; 1.1855x over previous
"""FFM layer kernel for Trainium2 (8 NeuronCores, data-parallel over batch).

Math (reference):
  idx[b,j]  = 13 + j*10000 + sparse_x[b,j]                 (26 sparse fields)
  linear    = dense_x @ w[:13] + sum_j w[idx] + w0         (B,1)
  field_f   = einsum('bd,dfk', dense_x, v[:13]) + sum_j v[idx]   (B,39,8)
  s         = sum_f field_f                                 (B,8)
  cross     = 0.5*(sum_k s^2 - sum_{f,k} field_f^2)
  out       = sigmoid(linear + cross)

Device strategy (per core, 512 samples):
  - Table rows augmented+padded on host in fp16: row r = [v[r].flat(312),
    w[r], 0...] (384 f16 = 768 B, %256==0 for dma_gather).  fp16 keeps the
    quantization error ~7e-3 (bf16 would be 8e-2, over the 2e-2 budget).
  - 26 dma_gather ops (one per sparse field, 512 int16 indices each) spread
    round-robin over 4 SWDGE queues so all 4 GPSIMD core pairs generate
    descriptors concurrently; each gather lands in its own SBUF tile so
    descriptor generation never stalls on accumulate progress.
  - DVE accumulates the landed tiles into 4 per-queue partial sums (the
    queue's first gather tile doubles as its accumulator) in fp16 (2x DVE
    rate), then combines pairwise.
  - Dense contribution (and the linear term + w0) via one PE matmul per chunk
    into a [128, 4, 512] PSUM tile (one bank per chunk): lhsT = dense_x^T
    chunk padded to [128,128] with an all-ones row 13, rhs[d] = [v[d].flat,
    w[d], 0...] in f32, rhs[13,312] = w0.
  - Final phase runs each op once across all 4 chunks: field add, square
    (ACT), strided reductions for sum_f / sum f^2, sigmoid (ACT), one store.
"""

import os
import numpy as np

N_DENSE = 13
N_SPARSE = 26
VOCAB = 10000
N_FIELD = 39
N_FEAT = N_DENSE + N_SPARSE * VOCAB  # 260013
K = 8
ROW = N_FIELD * K  # 312 v elems; w sits at col 312
USED = ROW + 1  # 313 meaningful row cols
ROWE = 384  # padded fp16 row -> 768 B (%256==0)
BATCH = 4096
N_CORES = 8
BC = BATCH // N_CORES  # 512 per core
P = 128
NCHUNK = BC // P  # 4
IDXC = BC // 16  # 32 int16 index columns per field
NQ = 4  # SWDGE queues / GPSIMD core pairs

_CACHE: dict = {}


def _build_program():
    import concourse.bacc as bacc
    import concourse.tile as tile
    import concourse.mybir as mybir

    f32 = mybir.dt.float32
    f16 = mybir.dt.float16
    i16 = mybir.dt.int16

    acc_f32 = os.environ.get("K_ACC", "f16") == "f32"

    nc = bacc.Bacc(
        "TRN2", target_bir_lowering=False, debug=False, num_swdge_queues=NQ
    )

    table = nc.dram_tensor("table", [N_FEAT, ROWE], f16, kind="ExternalInput")
    xt = nc.dram_tensor("xt", [P, BC], f32, kind="ExternalInput")
    vd = nc.dram_tensor("vd", [P, ROWE], f32, kind="ExternalInput")
    idx16 = nc.dram_tensor("idx16", [P, N_SPARSE * IDXC], i16, kind="ExternalInput")
    out = nc.dram_tensor("out", [BC, 1], f32, kind="ExternalOutput")

    with tile.TileContext(nc) as tc:
        with (
            tc.tile_pool(name="const", bufs=1) as cpool,
            tc.tile_pool(name="gather", bufs=N_SPARSE) as gpool,
            tc.tile_pool(name="work", bufs=1) as wpool,
            tc.tile_pool(name="psum", bufs=1, space="PSUM") as ppool,
        ):
            # index tiles first: the gathers depend only on these
            idx_sb = cpool.tile([P, N_SPARSE * IDXC], i16)
            nc.sync.dma_start(out=idx_sb[:], in_=idx16[:])
            # dense inputs on the other HWDGE queue; only the PE needs them
            xt_sb = cpool.tile([P, BC], f32)
            nc.scalar.dma_start(out=xt_sb[:], in_=xt[:])
            vd_sb = cpool.tile([P, ROWE], f32)
            nc.scalar.dma_start(out=vd_sb[:], in_=vd[:])

            # dense part: one matmul per chunk, each into its own PSUM bank
            psum = ppool.tile([P, NCHUNK, 512], f32, space="PSUM")
            for c in range(NCHUNK):
                nc.tensor.matmul(
                    out=psum[:, c, :ROWE],
                    lhsT=xt_sb[:, c * P:(c + 1) * P],
                    rhs=vd_sb[:],
                    start=True,
                    stop=True,
                )

            gtiles = []
            for j in range(N_SPARSE):
                g = gpool.tile([P, NCHUNK, ROWE], f16, tag="g", name=f"g{j}")
                gtiles.append(g)
                base = N_DENSE + j * VOCAB
                nc.gpsimd.dma_gather(
                    out_ap=g[:],
                    in_ap=table[base:base + VOCAB, :],
                    idxs_ap=idx_sb[:, j * IDXC:(j + 1) * IDXC],
                    num_idxs=BC,
                    num_idxs_reg=BC,
                    elem_size=ROWE,
                    single_packet=True,
                    queue_num=j % NQ,
                )
                if acc_f32:
                    if j == 0:
                        accs = [
                            cpool.tile([P, NCHUNK, 314], f32, name=f"acc{q}")
                            for q in range(NQ)
                        ]
                    q = j % NQ
                    if j < NQ:
                        nc.vector.tensor_copy(
                            out=accs[q][:], in_=g[:, :, :314]
                        )
                    else:
                        nc.vector.tensor_tensor(
                            out=accs[q][:], in0=accs[q][:], in1=g[:, :, :314],
                            op=mybir.AluOpType.add,
                        )
                elif j >= NQ:
                    # queue q's first gather tile is its fp16 accumulator
                    a = gtiles[j % NQ]
                    nc.vector.tensor_tensor(
                        out=a[:, :, :314], in0=a[:, :, :314],
                        in1=g[:, :, :314], op=mybir.AluOpType.add,
                    )

            # combine the per-queue partial sums
            if acc_f32:
                a0, a1, a2, a3 = accs
                sl = slice(None)
            else:
                a0, a1, a2, a3 = (gtiles[q][:, :, :314] for q in range(NQ))
                a0_t = gtiles[0]
            if acc_f32:
                nc.vector.tensor_tensor(out=a0[:], in0=a0[:], in1=a1[:],
                                        op=mybir.AluOpType.add)
                nc.vector.tensor_tensor(out=a2[:], in0=a2[:], in1=a3[:],
                                        op=mybir.AluOpType.add)
                nc.vector.tensor_tensor(out=a0[:], in0=a0[:], in1=a2[:],
                                        op=mybir.AluOpType.add)
                acc_ap = accs[0][:, :, :USED]
            else:
                nc.vector.tensor_tensor(out=a0, in0=a0, in1=a1,
                                        op=mybir.AluOpType.add)
                nc.vector.tensor_tensor(out=a2, in0=a2, in1=a3,
                                        op=mybir.AluOpType.add)
                nc.vector.tensor_tensor(out=a0, in0=a0, in1=a2,
                                        op=mybir.AluOpType.add)
                acc_ap = a0_t[:, :, :USED]

            # field[p, c, 0:313] = psum + acc   (col 312 = full linear term)
            field = wpool.tile([P, NCHUNK, 320], f32)
            nc.vector.tensor_tensor(
                out=field[:, :, :USED], in0=psum[:, :, :USED], in1=acc_ap,
                op=mybir.AluOpType.add,
            )

            # q = sum(field[:, :, :312]^2) per chunk
            sq = wpool.tile([P, NCHUNK, ROW], f32)
            nc.scalar.square(sq[:], field[:, :, :ROW])
            qs = wpool.tile([P, NCHUNK, 1], f32)
            nc.vector.reduce_sum(out=qs[:], in_=sq[:], axis=mybir.AxisListType.X)

            # s[c, k] = sum_f field[c, f*8+k]
            fv = field[:, :, :ROW].rearrange("p c (f k) -> p c k f", f=N_FIELD)
            s = wpool.tile([P, NCHUNK, K], f32)
            nc.vector.reduce_sum(out=s[:], in_=fv, axis=mybir.AxisListType.X)

            ss = wpool.tile([P, NCHUNK, K], f32)
            nc.vector.tensor_tensor(out=ss[:], in0=s[:], in1=s[:],
                                    op=mybir.AluOpType.mult)
            ssum = wpool.tile([P, NCHUNK, 1], f32)
            nc.vector.reduce_sum(out=ssum[:], in_=ss[:], axis=mybir.AxisListType.X)
            d = wpool.tile([P, NCHUNK, 1], f32)
            nc.vector.tensor_tensor(out=d[:], in0=ssum[:], in1=qs[:],
                                    op=mybir.AluOpType.subtract)
            # dd = 0.5*d + linear
            dd = wpool.tile([P, NCHUNK, 1], f32)
            nc.vector.scalar_tensor_tensor(
                out=dd[:], in0=d[:], scalar=0.5, in1=field[:, :, ROW:ROW + 1],
                op0=mybir.AluOpType.mult, op1=mybir.AluOpType.add,
            )
            oc = wpool.tile([P, NCHUNK], f32)
            nc.scalar.activation(
                oc[:], dd[:, :, 0], mybir.ActivationFunctionType.Sigmoid
            )
            nc.sync.dma_start(
                out=out[:].rearrange("(c p) one -> p (c one)", c=NCHUNK),
                in_=oc[:],
            )

    nc.compile()
    return nc


def _prep_inputs(dense_x, sparse_x, w0, w, v):
    table = np.zeros((N_FEAT, ROWE), dtype=np.float16)
    table[:, :ROW] = v.reshape(N_FEAT, ROW).astype(np.float16)
    table[:, ROW] = w[:, 0].astype(np.float16)

    vd = np.zeros((P, ROWE), dtype=np.float32)
    vd[:N_DENSE, :ROW] = v[:N_DENSE].reshape(N_DENSE, ROW)
    vd[:N_DENSE, ROW] = w[:N_DENSE, 0]
    vd[N_DENSE, ROW] = np.float32(w0[0])

    xt_full = np.zeros((P, BATCH), dtype=np.float32)
    xt_full[:N_DENSE] = dense_x.T
    xt_full[N_DENSE] = 1.0

    in_maps = []
    for r in range(N_CORES):
        b0 = r * BC
        sp = sparse_x[b0:b0 + BC].astype(np.int16)  # values < 10000 fit
        idx16 = np.zeros((P, N_SPARSE * IDXC), dtype=np.int16)
        for j in range(N_SPARSE):
            # gather position g reads idx[g%16, g//16]; g == sample index.
            # Replicated across all 8 GPSIMD-core partition groups (HW reads
            # its own 16-partition window).
            blk = sp[:, j].reshape(IDXC, 16).T
            idx16[:, j * IDXC:(j + 1) * IDXC] = np.tile(blk, (P // 16, 1))
        in_maps.append(
            {
                "table": table,
                "xt": np.ascontiguousarray(xt_full[:, b0:b0 + BC]),
                "vd": vd,
                "idx16": idx16,
            }
        )
    return in_maps


def kernel(dense_x, sparse_x, w0, w, v, _trace=False, _trace_kwargs=None):
    from concourse.bass_utils import run_bass_kernel_spmd

    if "nc" not in _CACHE:
        _CACHE["nc"] = _build_program()
    nc = _CACHE["nc"]

    in_maps = _prep_inputs(dense_x, sparse_x, w0, w, v)
    kw = {}
    if _trace:
        kw["trace"] = True
        if _trace_kwargs:
            kw.update(_trace_kwargs)
    res = run_bass_kernel_spmd(nc, in_maps, core_ids=list(range(N_CORES)), **kw)
    outs = [res.results[r]["out"] for r in range(N_CORES)]
    full = np.concatenate(outs, axis=0).astype(np.float32)
    if _trace:
        _CACHE["last_exec_time_ns"] = res.exec_time_ns
        _CACHE["last_results"] = res
    return full
